# revision 19
# baseline (speedup 1.0000x reference)
"""ViT self-attention (B=32, S=577, D=1024, H=16, Dh=64) on 8 TRN2 NeuronCores.

Sharding: data-parallel over batch — each core gets 4 batch elements, no
collectives. All matmul operands are bf16 (1 PE cycle/row vs 4 for fp32;
fp32 accumulation in PSUM), weights are loaded once per core and kept
resident in SBUF as bf16.

Per core, per batch:
  phase 0: PE-transpose X [577,1024] -> X^T stored as one bf16 tile
           [128, 8*577] (din-tile j at col block j)
  phase 1: Q^T = Wq^T X^T, K^T = Wk^T X^T (lhsT=W bf16, rhs=X^T bf16; bias
           folded into DVE evac, bf16 out), V natural = X Wv stored bf16
           with a ones column per head ([V_h | 1] -> softmax denominator
           comes out of the ctx matmul for free)
  phase 2: per head pair (row-packed K=64 matmuls at tile_position
           (0,0)/(64,0) run concurrently): S^T = K^T'Q^T; P^T = exp(S^T/8)
           on ACT (bf16); ctx = matmul(lhsT=P^T, rhs=[V_h|1]) accumulated
           in PSUM, denominator in col 64; DVE: recip + fused
           (ctx*recip + bv) evac.
  phase 3: DMA out per 128-token tile.
"""

import numpy as np

import concourse.bass as bass
import concourse.mybir as mybir
import concourse.tile as tile
from concourse.bass import ds, ts
from concourse.bass_utils import run_bass_kernel_spmd
from concourse.masks import make_identity

F32 = mybir.dt.float32
BF16 = mybir.dt.bfloat16

# ---------------------------------------------------------------------------
# Wait-legalization patch: this walrus build accepts at most ONE ge-mode sync
# wait per instruction (eq-mode counts as two). Tile's sem assignment attaches
# multi-waits directly to instructions, so hoist extras onto standalone
# EventSemaphore carriers (same engine queue, immediately preceding — identical
# semantics, queue is in-order).
# ---------------------------------------------------------------------------
_ctr = [0]


def _split_waits(insts):
    out = []
    for inst in insts:
        si = inst.sync_info
        if si is not None and si.on_wait:
            waits = list(si.on_wait)
            if len(waits) == 1 and waits[0].wait_mode != "sem-eq-imm":
                move = []
            else:
                move = waits
            for w in move:
                _ctr[0] += 1
                ev = mybir.InstEventSemaphore(
                    name=f"wsplit_{_ctr[0]}", opcode="EventSemaphore",
                    engine=inst.engine, debug=inst.debug, ins=[], outs=[],
                    sync_info=mybir.SyncInfo(on_wait=[w], on_update=[]),
                )
                out.append(ev)
            if move:
                inst.sync_info = mybir.SyncInfo(on_wait=[], on_update=list(si.on_update))
        out.append(inst)
    return out


def _install_waitfix():
    if getattr(tile.TileContext, "_waitfix_installed", False):
        return
    from concourse.vector_clock import ScopedClock

    orig_lower = tile.TileContext._lower_ordered_insts

    def patched_lower(self, ordered):
        for name in list(ordered.keys()):
            ordered[name] = _split_waits(ordered[name])
        return orig_lower(self, ordered)

    def patched_dab(self, tick_clock, wait_clock):
        nc = self.nc
        probe = nc.sync.nop(nofuse=True)
        wait_clock.add_sem_waits(probe.ins, ScopedClock({None: tick_clock.global_clock}))
        si = probe.ins.sync_info
        waits = list(si.on_wait) if si is not None else []
        probe.ins.sync_info = mybir.SyncInfo(
            on_wait=[], on_update=list(si.on_update) if si else []
        )
        for w in waits:
            _ctr[0] += 1
            ev = mybir.InstEventSemaphore(
                name=f"wsplit_dab_{_ctr[0]}", opcode="EventSemaphore",
                engine=mybir.EngineType.SP, debug=probe.ins.debug, ins=[], outs=[],
                sync_info=mybir.SyncInfo(on_wait=[w], on_update=[]),
            )
            nc.sync.add_instruction(ev)
        nc.sync.drain()
        nc.all_engine_barrier()
        assert self.sems is not None
        popped = nc._tile_sem_poison_stack.pop()
        assert popped is self._sem_poison
        nc.clear_and_free_semaphores(list(self.sems.allocated().values()))
        nc.all_engine_barrier()

    tile.TileContext._lower_ordered_insts = patched_lower
    tile.TileContext._drain_and_barrier = patched_dab
    tile.TileContext._waitfix_installed = True


_install_waitfix()

N_CORES = 8
B, S, D = 32, 577, 1024
H, Dh = 16, 64
BPC = B // N_CORES  # batches per core
S_TILES = [(t * 128, min(128, S - t * 128)) for t in range((S + 127) // 128)]  # 5
NT = len(S_TILES)
ND = D // 128  # 8 din/dout tiles
HPAIRS = H // 2
VW = H * 65  # v row width per token tile (16 heads x [64 | 1])

AF = mybir.ActivationFunctionType
OP = mybir.AluOpType


def _view(ap, offset, pattern):
    """AP with the same partition dim but custom free-dim pattern."""
    return bass.AP(
        tensor=ap.tensor, offset=ap.offset + offset, ap=[ap.ap[0]] + pattern
    )


def build_nc(niter=1):
    nc = bass.Bass()
    hidden = nc.declare_dram_parameter("hidden", [BPC, S, D], F32, isOutput=False)
    wq = nc.declare_dram_parameter("Wq", [D, D], F32, isOutput=False)
    bq = nc.declare_dram_parameter("bq", [D], F32, isOutput=False)
    wk = nc.declare_dram_parameter("Wk", [D, D], F32, isOutput=False)
    bk = nc.declare_dram_parameter("bk", [D], F32, isOutput=False)
    wv = nc.declare_dram_parameter("Wv", [D, D], F32, isOutput=False)
    bv = nc.declare_dram_parameter("bv", [D], F32, isOutput=False)
    out = nc.declare_dram_parameter("out", [BPC, S, D], F32, isOutput=True)

    with tile.TileContext(nc) as tc:
        with (
            tc.tile_pool(name="singles", bufs=1) as singles,
            tc.tile_pool(name="wstage", bufs=3) as wstage,
            tc.tile_pool(name="w", bufs=1) as w_pool,
            tc.tile_pool(name="xnat", bufs=3) as xnat_pool,
            tc.tile_pool(name="xb", bufs=2) as xb_pool,
            tc.tile_pool(name="xt", bufs=2) as xt_pool,
            tc.tile_pool(name="qkt", bufs=2) as qkt_pool,
            tc.tile_pool(name="v", bufs=2) as v_pool,
            tc.tile_pool(name="pT", bufs=2) as pT_pool,
            tc.tile_pool(name="ostage", bufs=1) as o_pool,
            tc.tile_pool(name="rc", bufs=8) as rc_pool,
            tc.tile_pool(name="psmm", bufs=2, space="PSUM") as ps_mm,
            tc.tile_pool(name="pssc", bufs=2, space="PSUM") as ps_sc,
            tc.tile_pool(name="psctx", bufs=2, space="PSUM") as ps_ctx,
        ):
            # --- constants ---
            identity = singles.tile([128, 128], F32)
            make_identity(nc, identity)
            identity_bf = singles.tile([128, 128], BF16)
            nc.gpsimd.tensor_copy(out=identity_bf, in_=identity)
            # per-dout-tile bias columns: bqt[:, m] = bq[128m : 128(m+1)]
            bqt = singles.tile([128, ND], F32)
            bkt = singles.tile([128, ND], F32)
            nc.gpsimd.dma_start(out=bqt, in_=bq[:].rearrange("(m p) -> p m", p=128))
            nc.gpsimd.dma_start(out=bkt, in_=bk[:].rearrange("(m p) -> p m", p=128))
            # bv broadcast to all 128 partitions
            bvb = singles.tile([128, D], F32)
            bv_ap = bv[:]
            nc.gpsimd.dma_start(
                out=bvb,
                in_=bass.AP(tensor=bv_ap.tensor, offset=bv_ap.offset, ap=[[0, 128]] + bv_ap.ap),
            )

            wts = {}

            def load_weights():
                # load once, downcast to bf16, keep resident. Emitted after
                # batch-0 transposes so the first hidden DMAs go out first;
                # order q,k,v matches pair-gating (pairs need全 wq+wk, V lags).
                for wname, wdram in (("q", wq), ("k", wk), ("v", wv)):
                    tiles = []
                    for k in range(ND):
                        stg = wstage.tile([128, D], F32, tag="wstage")
                        nc.gpsimd.dma_start(out=stg, in_=wdram[ts(k, 128), :])
                        wt = w_pool.tile([128, D], BF16, tag=f"w{wname}{k}")
                        nc.vector.tensor_copy(out=wt, in_=stg)
                        tiles.append(wt)
                    wts[wname] = tiles

            def make_trans(b):
                """Phase 0 for batch b: returns (xt_tile, chunk_fn(t))."""
                xt = xt_pool.tile([128, ND * S], BF16, tag="xt", name="xt")

                def chunk(t):
                    t0, st = S_TILES[t]
                    xn = xnat_pool.tile([128, D], F32, tag="xn")
                    nc.sync.dma_start(out=xn[:st], in_=hidden[b, t0 : t0 + st, :])
                    xb = xb_pool.tile([128, D], BF16, tag="xb")
                    nc.gpsimd.tensor_copy(out=xb[:st], in_=xn[:st])
                    # all 8 bf16 transposes fit one 1-bank psum tile
                    pst = ps_mm.tile([128, 1024], BF16, tag="mm", name="psmm")
                    for j in range(ND):
                        nc.tensor.transpose(
                            pst[:, 128 * j : 128 * j + st],
                            xb[:st, ts(j, 128)],
                            identity_bf[:st, :st],
                        )
                    # one evac: psum cols (j,tok) -> xt cols j*S + t0 + tok
                    nc.vector.tensor_copy(
                        out=_view(xt[:], t0, [[S, ND], [1, st]]),
                        in_=_view(pst[:], 0, [[128, ND], [1, st]]),
                    )

                return xt, chunk

            def make_v(xt):
                """V projection: returns (vt_tile, chunk_fn(t))."""
                vt = v_pool.tile([128, NT * VW], BF16, tag="vt", name="vt")

                # ones columns for the whole tile in one memset
                nc.vector.memset(_view(vt[:], 64, [[65, H * NT], [1, 1]]), 1.0)

                def chunk(t):
                    t0, st = S_TILES[t]
                    for half in range(2):
                        ps = ps_mm.tile([128, 512], F32, tag="mm", name="psmm")
                        for k in range(ND):
                            lhs = xt[:, k * S + t0 : k * S + t0 + st]
                            nc.tensor.matmul(
                                ps[:st, 0:512], lhs,
                                wts["v"][k][:, 512 * half : 512 * half + 512],
                                start=(k == 0), stop=(k == ND - 1),
                            )
                        # heads 8*half .. 8*half+7: psum (h,c) -> vt cols
                        # t*VW + 65*(8*half+h) + c
                        nc.vector.tensor_copy(
                            out=_view(
                                vt[:], t * VW + 65 * 8 * half, [[65, 8], [1, 64]]
                            )[:st],
                            in_=ps[:st].rearrange("p (h c) -> p h c", c=64),
                        )

                return vt, chunk

            def emit_pair(p, xt, vt, ost):
                """Project Q^T/K^T for dout tile p, then scores+exp+ctx for
                heads 2p, 2p+1."""
                qp = qkt_pool.tile([128, S], BF16, tag=f"qt{p}", name=f"qt{p}")
                kp = qkt_pool.tile([128, S], BF16, tag=f"kt{p}", name=f"kt{p}")
                for wtiles, dst, bias_t in (
                    (wts["q"], qp, bqt),
                    (wts["k"], kp, bkt),
                ):
                    psa = ps_mm.tile([128, 512], F32, tag="mm", name="psmm")
                    psb = ps_mm.tile([128, 512], F32, tag="mm", name="psmm")
                    for k in range(ND):
                        lhs = wtiles[k][:, ts(p, 128)]
                        nc.tensor.matmul(
                            psa[:, 0:512], lhs, xt[:, k * S : k * S + 512],
                            start=(k == 0), stop=(k == ND - 1),
                        )
                        nc.tensor.matmul(
                            psb[:, 0 : S - 512], lhs, xt[:, k * S + 512 : k * S + S],
                            start=(k == 0), stop=(k == ND - 1),
                        )
                    nc.vector.tensor_scalar_add(
                        dst[:, 0:512], psa[:, 0:512], bias_t[:, p : p + 1]
                    )
                    nc.vector.tensor_scalar_add(
                        dst[:, 512:S], psb[:, 0 : S - 512], bias_t[:, p : p + 1]
                    )
                pt = pT_pool.tile([128, 2 * NT * S], BF16, tag="pT", name="pT")
                for t, (t0, st) in enumerate(S_TILES):
                    for half in range(2):
                        h0 = half * 64
                        psS = ps_sc.tile([128, 1024], F32, tag="sc", name="pssc")
                        lhs = kp[h0 : h0 + 64, t0 : t0 + st]
                        nc.tensor.matmul(
                            psS[:st, 0:512], lhs, qp[h0 : h0 + 64, 0:512],
                            start=True, stop=True, tile_position=(h0, 0),
                        )
                        nc.tensor.matmul(
                            psS[:st, 512:S], lhs, qp[h0 : h0 + 64, 512:S],
                            start=True, stop=True, tile_position=(h0, 0),
                        )
                        off = (2 * t + half) * S
                        nc.scalar.activation(
                            pt[:st, off : off + S], psS[:st, 0:S],
                            AF.Exp, scale=0.125,
                        )
                for half in range(2):
                    h = 2 * p + half
                    psc = ps_ctx.tile([128, 512], F32, tag="ctx", name="psctx")
                    for j, (j0, sj) in enumerate(S_TILES):
                        for t, (t0, st) in enumerate(S_TILES):
                            off = (2 * t + half) * S
                            nc.tensor.matmul(
                                psc[:sj, ds(65 * j, 65)],
                                pt[:st, off + j0 : off + j0 + sj],
                                vt[:st, VW * t + 65 * h : VW * t + 65 * h + 65],
                                start=(t == 0), stop=(t == NT - 1),
                            )
                    # reciprocal of the denominator columns (j=4 tile has
                    # only 65 valid rows, so split it off)
                    rc = rc_pool.tile([128, NT], F32, tag="rc", name="rc")
                    nc.vector.reciprocal(
                        rc[:, 0 : NT - 1],
                        _view(psc[:], 64, [[65, NT - 1], [1, 1]]),
                    )
                    lsj = S_TILES[NT - 1][1]
                    nc.vector.reciprocal(
                        rc[:lsj, NT - 1 : NT],
                        psc[:lsj, ds(65 * (NT - 1) + 64, 1)],
                    )
                    for j, (j0, sj) in enumerate(S_TILES):
                        nc.vector.scalar_tensor_tensor(
                            out=ost[:sj, 1024 * j + 64 * h : 1024 * j + 64 * h + 64],
                            in0=psc[:sj, ds(65 * j, 64)],
                            scalar=rc[:sj, j : j + 1],
                            in1=bvb[:sj, ds(64 * h, 64)],
                            op0=OP.mult,
                            op1=OP.add,
                        )

            # Software-pipelined emission: while batch i's attention pairs
            # run (ACT-heavy), emit batch i+1's transposes + V projection
            # between pairs so PE always has independent work.
            steps = [(it, b) for it in range(niter) for b in range(BPC)]
            # after pair p, emit these (kind, t) chunks of the NEXT batch
            PLAN = {0: [("t", 0)], 1: [("t", 1), ("v", 0)], 2: [("t", 2), ("v", 1)],
                    3: [("t", 3), ("v", 2)], 4: [("t", 4), ("v", 3)], 5: [("v", 4)]}

            xt0, tchunk = make_trans(steps[0][1])
            for t in range(NT):
                tchunk(t)
            load_weights()
            cur_xt = xt0
            cur_vt, cur_vchunk = make_v(xt0)
            for t in range(NT):
                cur_vchunk(t)
            for i, (it, b) in enumerate(steps):
                if i + 1 < len(steps):
                    nxt_xt, nxt_tchunk = make_trans(steps[i + 1][1])
                    nxt_vchunk = None
                    nxt_vt = None
                ost = o_pool.tile([128, NT * 1024], F32, tag="ost", name="ost")
                for p in range(HPAIRS):
                    emit_pair(p, cur_xt, cur_vt, ost)
                    if i + 1 < len(steps):
                        for kind, t in PLAN.get(p, []):
                            if kind == "t":
                                nxt_tchunk(t)
                            else:
                                if nxt_vchunk is None:
                                    nxt_vt, nxt_vchunk = make_v(nxt_xt)
                                nxt_vchunk(t)
                for j, (j0, sj) in enumerate(S_TILES):
                    nc.sync.dma_start(
                        out=out[b, j0 : j0 + sj, :],
                        in_=ost[:sj, 1024 * j : 1024 * (j + 1)],
                    )
                if i + 1 < len(steps):
                    cur_xt, cur_vt = nxt_xt, nxt_vt

    return nc


_NC = None


def kernel(hidden_states, Wq, bq, Wk, bk, Wv, bv):
    global _NC
    if _NC is None:
        _NC = build_nc()
    hs = np.ascontiguousarray(np.asarray(hidden_states, dtype=np.float32))
    args = {
        "Wq": np.ascontiguousarray(np.asarray(Wq, np.float32)),
        "bq": np.ascontiguousarray(np.asarray(bq, np.float32)),
        "Wk": np.ascontiguousarray(np.asarray(Wk, np.float32)),
        "bk": np.ascontiguousarray(np.asarray(bk, np.float32)),
        "Wv": np.ascontiguousarray(np.asarray(Wv, np.float32)),
        "bv": np.ascontiguousarray(np.asarray(bv, np.float32)),
    }
    in_maps = [
        {"hidden": hs[i * BPC : (i + 1) * BPC], **args} for i in range(N_CORES)
    ]
    res = run_bass_kernel_spmd(_NC, in_maps, list(range(N_CORES)))
    return np.concatenate([res.results[i]["out"] for i in range(N_CORES)], axis=0)
